# revision 3
# baseline (speedup 1.0000x reference)
"""ContrastHead KNN loss on 8 TRN2 cores — v3 "stream-expand".

v2's bottleneck was GpSimd descriptor generation (~5.5 ns/desc x 134k
descs = 764 us).  v3 removes descriptors from the main path entirely:

  - Slots are sorted by neighbor row.  Each row's first F=6 slots form the
    "main" grid [100352 rows x 6]; the device STREAMS the bf16 table
    sequentially (12.8 MB, large descriptors, no SWDGE) and expands each
    row to its 6 slots with a stride-0 broadcast AP on the DVE.
  - d2 = |g|^2 + |p|^2 - 2 g.p: the device computes only the cross term
    (mult + bf16 half-tree + f32 reduce); row norms are added on the host.
  - Per-slot point rows are staged by the host as a sequential bf16 stream.
  - Overflow slots (rank >= 6, ~29k/core = 6.6%) go through a small
    dma_gather spill path (256B descs on the plain bf16 table, 2 chunks).
  - Host: un-permute, add norms, sqrt/softmax/mask (O(M*K) numpy).
"""
import numpy as np
import ml_dtypes

M_TOTAL = 100000
C = 64
K = 35
N_CORES = 8
M_CORE = M_TOTAL // N_CORES          # 12500

F = 6                                # padded slots per row (main path)
TR = 16                              # rows per partition per tile
TILE_ROWS = 128 * TR                 # 2048
NTILES = 49
ROWS_PAD = NTILES * TILE_ROWS        # 100352

L = 1024                             # idx per spill gather call
SPC = [21, 12]                       # spill calls per chunk (rows <65536, >=)
SCALLS = sum(SPC)                    # 33

_EPS = 1e-7
TEMPERATURE = 0.1
WEIGHT = 1.0

_cached = {}


def _get_nc():
    if "nc" in _cached:
        return _cached["nc"]
    import concourse.bacc as bacc
    import concourse.mybir as mybir
    import concourse.tile as tile
    import bass_rust
    from concourse.vector_clock import ScopedClock

    def _patched_drain_and_barrier(self, tick_clock, wait_clock):
        holder = self.nc.sync.nop(nofuse=True, hint="tile_exit_waits")
        wait_clock.add_sem_waits(
            holder.ins, ScopedClock({None: tick_clock.global_clock})
        )
        si = holder.ins.sync_info
        waits = list(si.on_wait) if si is not None else []
        if len(waits) > 1:
            si.on_wait[:] = waits[:1]
            for w in waits[1:]:
                nop = self.nc.sync.nop(nofuse=True, hint="tile_exit_waits")
                nop.ins.sync_info = mybir.SyncInfo(on_wait=[w], on_update=[])
        self.nc.sync.drain()
        self.nc.all_engine_barrier()
        assert self.sems is not None
        popped = self.nc._tile_sem_poison_stack.pop()
        assert popped is self._sem_poison
        self.nc.clear_and_free_semaphores(list(self.sems.allocated().values()))
        self.nc.all_engine_barrier()

    tile.TileContext._drain_and_barrier = _patched_drain_and_barrier

    def _split_multi_waits(nc, limit=1):
        counter = [0]
        for func in nc.m.functions:
            for bb in func.blocks:
                out = []
                changed = False
                for inst in bb.instructions:
                    si = inst.sync_info
                    waits = list(si.on_wait) if si is not None else []
                    if len(waits) > limit:
                        for w in waits[:-limit]:
                            nop = bass_rust.InstNoOp(
                                name=f"waitsplit-nop-{counter[0]}", ins=[], outs=[]
                            )
                            counter[0] += 1
                            nop.engine = inst.engine
                            nop.sync_info = mybir.SyncInfo(on_wait=[w], on_update=[])
                            nop.bass_nofuse = True
                            out.append(nop)
                        inst.sync_info = mybir.SyncInfo(
                            on_wait=waits[-limit:], on_update=list(si.on_update)
                        )
                        changed = True
                    out.append(inst)
                if changed:
                    bb.instructions = out

    nc = bacc.Bacc(
        "TRN2", target_bir_lowering=False, debug=False, num_swdge_queues=4
    )
    f32 = mybir.dt.float32
    bf16 = mybir.dt.bfloat16
    i16 = mybir.dt.int16

    tab_d = nc.dram_tensor("table", [ROWS_PAD, C], bf16, kind="ExternalInput")
    pmain_d = nc.dram_tensor(
        "pmain", [NTILES, 128, TR * F * C], bf16, kind="ExternalInput"
    )
    d2m_d = nc.dram_tensor("d2m", [NTILES, 128, TR * F], f32, kind="ExternalOutput")
    sidx_d = nc.dram_tensor("sidx", [SCALLS, 128, L // 16], i16, kind="ExternalInput")
    spexp_d = nc.dram_tensor(
        "spexp", [SCALLS, 128, (L // 128) * 128], bf16, kind="ExternalInput"
    )
    d2s_d = nc.dram_tensor(
        "d2s", [SCALLS, 128, (L // 128) * 2], f32, kind="ExternalOutput"
    )

    # spill gather windows: idx*256B = row 2*idx, elem = 2 rows (256B)
    spill_aps = []
    for base_row, wrows in ((0, 65536 + 2), (65536, ROWS_PAD - 65536)):
        sl = tab_d[base_row : base_row + wrows, :]
        ia = sl.copy()
        ia.ap = type(ia.ap)([[128, (wrows - 2) // 2 + 1], [1, 128]])
        spill_aps.append(ia)
    scall_chunk = [0] * SPC[0] + [1] * SPC[1]

    PCs = L // 128  # 8 descs per partition per spill call

    with tile.TileContext(nc) as tc:
        with (
            tc.tile_pool(name="ix", bufs=4) as ix_pool,
            tc.tile_pool(name="sp", bufs=3) as sp_pool,
            tc.tile_pool(name="tab", bufs=3) as tab_pool,
            tc.tile_pool(name="pm", bufs=3) as pm_pool,
            tc.tile_pool(name="wk", bufs=3) as wk_pool,
            tc.tile_pool(name="out", bufs=3) as out_pool,
        ):
            # ---- spill path first (keeps Pool/queues busy early) ----
            for s in range(SCALLS):
                ch = scall_chunk[s]
                it = ix_pool.tile([128, L // 16], i16)
                nc.sync.dma_start(out=it[:], in_=sidx_d[s, :, :])
                spt = sp_pool.tile([128, PCs * 128], bf16, tag="sg")
                nc.gpsimd.dma_gather(
                    out_ap=spt[:].rearrange("p (a b) -> p a b", b=128),
                    in_ap=spill_aps[ch],
                    idxs_ap=it[:],
                    num_idxs=L,
                    num_idxs_reg=L,
                    elem_size=128,
                    elem_step=128,
                    queue_num=s % 4,
                )
                spp = sp_pool.tile([128, PCs * 128], bf16, tag="sp")
                nc.sync.dma_start(out=spp[:], in_=spexp_d[s, :, :])
                sx = sp_pool.tile([128, PCs * 128], bf16, tag="sx")
                nc.vector.tensor_tensor(
                    out=sx[:], in0=spt[:], in1=spp[:], op=mybir.AluOpType.mult
                )
                v0 = sx[:].rearrange("p (a b) -> p a b", b=64)     # [128,16,64]
                st1 = sp_pool.tile([128, PCs * 64], bf16, tag="st1")
                nc.vector.tensor_tensor(
                    out=st1[:].rearrange("p (a b) -> p a b", b=32),
                    in0=v0[:, :, 0:32],
                    in1=v0[:, :, 32:64],
                    op=mybir.AluOpType.add,
                )
                v1 = st1[:].rearrange("p (a b) -> p a b", b=32)
                st2 = sp_pool.tile([128, PCs * 32], bf16, tag="st2")
                nc.vector.tensor_tensor(
                    out=st2[:].rearrange("p (a b) -> p a b", b=16),
                    in0=v1[:, :, 0:16],
                    in1=v1[:, :, 16:32],
                    op=mybir.AluOpType.add,
                )
                v2 = st2[:].rearrange("p (a b) -> p a b", b=16)
                st3 = sp_pool.tile([128, PCs * 16], bf16, tag="st3")
                nc.vector.tensor_tensor(
                    out=st3[:].rearrange("p (a b) -> p a b", b=8),
                    in0=v2[:, :, 0:8],
                    in1=v2[:, :, 8:16],
                    op=mybir.AluOpType.add,
                )
                so = out_pool.tile([128, PCs * 2], f32, tag="so")
                nc.vector.tensor_reduce(
                    out=so[:],
                    in_=st3[:].rearrange("p (a b) -> p a b", b=8),
                    axis=mybir.AxisListType.X,
                    op=mybir.AluOpType.add,
                )
                nc.sync.dma_start(out=d2s_d[s, :, :], in_=so[:])

            # ---- main stream-expand path ----
            for T in range(NTILES):
                tt = tab_pool.tile([128, TR * C], bf16)
                nc.sync.dma_start(
                    out=tt[:],
                    in_=tab_d[T * TILE_ROWS : (T + 1) * TILE_ROWS, :].rearrange(
                        "(p l) c -> p (l c)", p=128
                    ),
                )
                pm = pm_pool.tile([128, TR * F * C], bf16)
                nc.sync.dma_start(out=pm[:], in_=pmain_d[T, :, :])
                # broadcast AP: [part, TR, F(stride 0), C]
                t3 = tt[:].rearrange("p (l c) -> p l c", c=C)
                bap = t3.copy()
                pst = t3.ap.to_list()[0]
                bap.ap = type(bap.ap)([list(pst), [C, TR], [0, F], [1, C]])
                x = wk_pool.tile([128, TR * F * C], bf16, tag="x")
                nc.vector.tensor_tensor(
                    out=x[:].rearrange("p (l f c) -> p l f c", f=F, c=C),
                    in0=bap,
                    in1=pm[:].rearrange("p (l f c) -> p l f c", f=F, c=C),
                    op=mybir.AluOpType.mult,
                )
                w0 = x[:].rearrange("p (a b) -> p a b", b=64)      # [128,96,64]
                t1 = wk_pool.tile([128, TR * F * 32], bf16, tag="t1")
                nc.vector.tensor_tensor(
                    out=t1[:].rearrange("p (a b) -> p a b", b=32),
                    in0=w0[:, :, 0:32],
                    in1=w0[:, :, 32:64],
                    op=mybir.AluOpType.add,
                )
                w1 = t1[:].rearrange("p (a b) -> p a b", b=32)
                t2 = wk_pool.tile([128, TR * F * 16], bf16, tag="t2")
                nc.vector.tensor_tensor(
                    out=t2[:].rearrange("p (a b) -> p a b", b=16),
                    in0=w1[:, :, 0:16],
                    in1=w1[:, :, 16:32],
                    op=mybir.AluOpType.add,
                )
                w2 = t2[:].rearrange("p (a b) -> p a b", b=16)
                t3s = wk_pool.tile([128, TR * F * 8], bf16, tag="t3")
                nc.vector.tensor_tensor(
                    out=t3s[:].rearrange("p (a b) -> p a b", b=8),
                    in0=w2[:, :, 0:8],
                    in1=w2[:, :, 8:16],
                    op=mybir.AluOpType.add,
                )
                ot = out_pool.tile([128, TR * F], f32, tag="mo")
                nc.vector.tensor_reduce(
                    out=ot[:],
                    in_=t3s[:].rearrange("p (a b) -> p a b", b=8),
                    axis=mybir.AxisListType.X,
                    op=mybir.AluOpType.add,
                )
                nc.sync.dma_start(out=d2m_d[T, :, :], in_=ot[:])

    nc.compile()
    _split_multi_waits(nc)
    _cached["nc"] = nc
    return nc


def _wrap16(arr):
    G, N = arr.shape
    w = arr.reshape(G, N // 16, 16).transpose(0, 2, 1)
    return np.ascontiguousarray(np.tile(w, (1, 8, 1)))


def _prep_core(flat, feats_bf, m0):
    """flat: [437500] neighbor rows in slot order. Returns device inputs +
    maps. Main: rows x F grid. Spill: rank>=F slots via 256B desc gather."""
    bf = ml_dtypes.bfloat16
    N = flat.size
    order = np.argsort(flat, kind="stable").astype(np.int64)
    srt = flat[order]
    cnt = np.bincount(flat, minlength=M_TOTAL).astype(np.int64)
    row_off = np.concatenate([[0], np.cumsum(cnt)])
    rank = np.arange(N) - row_off[srt]

    main = rank < F
    mm = np.full((ROWS_PAD, F), -1, np.int64)
    mm[srt[main], rank[main]] = order[main]

    pmain = np.zeros((ROWS_PAD * F, C), bf)
    mv = mm.ravel()
    val = mv >= 0
    pmain[val] = feats_bf[m0 + (mv[val] // K)]
    pmain = pmain.reshape(NTILES, 128, TR * F * C)

    # ---- spill ----
    sp_slots = order[~main]
    sp_rows = srt[~main]                       # sorted ascending
    sidx_all = (sp_rows // 2).astype(np.int64)
    half = (sp_rows % 2).astype(np.int64)
    chunk = (sp_rows >= 65536).astype(np.int64)
    idx_grid = np.zeros(SCALLS * L, np.int16)
    sgrid = np.full((SCALLS * L, 2), -1, np.int64)
    bounds = np.searchsorted(chunk, [0, 1, 2])
    pos = 0
    for ch in range(2):
        lo, hi = bounds[ch], bounds[ch + 1]
        n = hi - lo
        cap = SPC[ch] * L
        assert n <= cap, f"spill chunk {ch} overflow: {n} > {cap}"
        idx_grid[pos : pos + n] = (
            sidx_all[lo:hi] - 32768 * ch
        ).astype(np.int16)
        sgrid[pos + np.arange(n), half[lo:hi]] = sp_slots[lo:hi]
        pos += cap
    assert pos == SCALLS * L

    spexp = np.zeros((SCALLS * L * 2, C), bf)
    sv = sgrid.ravel()
    vals = sv >= 0
    spexp[vals] = feats_bf[m0 + (sv[vals] // K)]
    spexp = (
        spexp.reshape(SCALLS, L // 128, 128, 2 * C)
        .transpose(0, 2, 1, 3)
        .reshape(SCALLS, 128, (L // 128) * 128)
    )
    sidx = _wrap16(idx_grid.reshape(SCALLS, L))
    return sidx, spexp, pmain, mm, sgrid


def kernel(features, labels, neighbor_idx):
    from concourse.bass_utils import run_bass_kernel_spmd

    bf = ml_dtypes.bfloat16
    features = np.ascontiguousarray(np.asarray(features), dtype=np.float32)
    labels = np.asarray(labels).astype(np.int64)
    neighbor_idx = np.asarray(neighbor_idx).astype(np.int64)

    nc = _get_nc()

    feats_bf = features.astype(bf)
    tab = np.zeros((ROWS_PAD, C), bf)
    tab[:M_TOTAL] = feats_bf
    norms = (feats_bf.astype(np.float32) ** 2).sum(1)     # [100000]

    in_maps = []
    maps = []
    for c in range(N_CORES):
        m0 = c * M_CORE
        flat = neighbor_idx[m0 : m0 + M_CORE].ravel()
        sidx, spexp, pmain, mm, sgrid = _prep_core(flat, feats_bf, m0)
        maps.append((mm, sgrid, flat))
        in_maps.append(
            {"table": tab, "pmain": pmain, "sidx": sidx, "spexp": spexp}
        )
    _cached["in_maps"] = in_maps

    res = run_bass_kernel_spmd(nc, in_maps, list(range(N_CORES))).results

    posmask = (labels[:, None] == labels[neighbor_idx]).astype(np.float32)
    cnt = posmask.sum(-1)
    pm = ((cnt > 0) & (cnt < K)).astype(np.float32)

    loss_num = 0.0
    for c in range(N_CORES):
        mm, sgrid, flat = maps[c]
        m0 = c * M_CORE
        cross = np.empty(M_CORE * K, np.float32)
        d2m = res[c]["d2m"].reshape(ROWS_PAD, F)
        mv = mm.ravel()
        val = mv >= 0
        cross[mv[val]] = d2m.ravel()[val]
        d2s = (
            res[c]["d2s"]
            .reshape(SCALLS, 128, L // 128, 2)
            .transpose(0, 2, 1, 3)
            .reshape(SCALLS * L, 2)
        )
        sv = sgrid.ravel()
        vals = sv >= 0
        cross[sv[vals]] = d2s.ravel()[vals]

        slots_m = np.repeat(np.arange(M_CORE), K)
        d2_grid = (
            norms[flat] + norms[m0 + slots_m] - 2.0 * cross
        ).reshape(M_CORE, K)
        d2_grid = np.maximum(d2_grid, 0.0)

        dist = np.sqrt(d2_grid + _EPS)
        d = -dist
        d = d - d.max(axis=-1, keepdims=True)
        d = d / TEMPERATURE
        ex = np.exp(d)
        pos = (ex * posmask[m0 : m0 + M_CORE]).sum(-1)
        neg = ex.sum(-1)
        loss = -np.log(pos / neg + _EPS)
        loss_num += float((loss * pm[m0 : m0 + M_CORE]).sum())

    denom = max(float(pm.sum()), 1.0)
    return np.float32(loss_num / denom * WEIGHT)


# revision 4
# speedup vs baseline: 1.0015x; 1.0015x over previous
"""ContrastHead KNN loss on 8 TRN2 cores — v3 "stream-expand".

v2's bottleneck was GpSimd descriptor generation (~5.5 ns/desc x 134k
descs = 764 us).  v3 removes descriptors from the main path entirely:

  - Slots are sorted by neighbor row.  Each row's first F=6 slots form the
    "main" grid [100352 rows x 6]; the device STREAMS the bf16 table
    sequentially (12.8 MB, large descriptors, no SWDGE) and expands each
    row to its 6 slots with a stride-0 broadcast AP on the DVE.
  - d2 = |g|^2 + |p|^2 - 2 g.p: the device computes only the cross term
    (mult + bf16 half-tree + f32 reduce); row norms are added on the host.
  - Per-slot point rows are staged by the host as a sequential bf16 stream.
  - Overflow slots (rank >= 6, ~29k/core = 6.6%) go through a small
    dma_gather spill path (256B descs on the plain bf16 table, 2 chunks).
  - Host: un-permute, add norms, sqrt/softmax/mask (O(M*K) numpy).
"""
import numpy as np
import ml_dtypes

M_TOTAL = 100000
C = 64
K = 35
N_CORES = 8
M_CORE = M_TOTAL // N_CORES          # 12500

F = 6                                # padded slots per row (main path)
TR = 16                              # rows per partition per tile
TILE_ROWS = 128 * TR                 # 2048
NTILES = 49
ROWS_PAD = NTILES * TILE_ROWS        # 100352

L = 1024                             # idx per spill gather call
SPC = [21, 12]                       # spill calls per chunk (rows <65536, >=)
SCALLS = sum(SPC)                    # 33

_EPS = 1e-7
TEMPERATURE = 0.1
WEIGHT = 1.0

_cached = {}


def _get_nc():
    if "nc" in _cached:
        return _cached["nc"]
    import concourse.bacc as bacc
    import concourse.mybir as mybir
    import concourse.tile as tile
    import bass_rust
    from concourse.vector_clock import ScopedClock

    def _patched_drain_and_barrier(self, tick_clock, wait_clock):
        holder = self.nc.sync.nop(nofuse=True, hint="tile_exit_waits")
        wait_clock.add_sem_waits(
            holder.ins, ScopedClock({None: tick_clock.global_clock})
        )
        si = holder.ins.sync_info
        waits = list(si.on_wait) if si is not None else []
        if len(waits) > 1:
            si.on_wait[:] = waits[:1]
            for w in waits[1:]:
                nop = self.nc.sync.nop(nofuse=True, hint="tile_exit_waits")
                nop.ins.sync_info = mybir.SyncInfo(on_wait=[w], on_update=[])
        self.nc.sync.drain()
        self.nc.all_engine_barrier()
        assert self.sems is not None
        popped = self.nc._tile_sem_poison_stack.pop()
        assert popped is self._sem_poison
        self.nc.clear_and_free_semaphores(list(self.sems.allocated().values()))
        self.nc.all_engine_barrier()

    tile.TileContext._drain_and_barrier = _patched_drain_and_barrier

    def _split_multi_waits(nc, limit=1):
        counter = [0]
        for func in nc.m.functions:
            for bb in func.blocks:
                out = []
                changed = False
                for inst in bb.instructions:
                    si = inst.sync_info
                    waits = list(si.on_wait) if si is not None else []
                    if len(waits) > limit:
                        for w in waits[:-limit]:
                            nop = bass_rust.InstNoOp(
                                name=f"waitsplit-nop-{counter[0]}", ins=[], outs=[]
                            )
                            counter[0] += 1
                            nop.engine = inst.engine
                            nop.sync_info = mybir.SyncInfo(on_wait=[w], on_update=[])
                            nop.bass_nofuse = True
                            out.append(nop)
                        inst.sync_info = mybir.SyncInfo(
                            on_wait=waits[-limit:], on_update=list(si.on_update)
                        )
                        changed = True
                    out.append(inst)
                if changed:
                    bb.instructions = out

    nc = bacc.Bacc(
        "TRN2", target_bir_lowering=False, debug=False, num_swdge_queues=4
    )
    f32 = mybir.dt.float32
    bf16 = mybir.dt.bfloat16
    i16 = mybir.dt.int16

    tab_d = nc.dram_tensor("table", [ROWS_PAD, C], bf16, kind="ExternalInput")
    pmain_d = nc.dram_tensor(
        "pmain", [NTILES, 128, TR * F * C], bf16, kind="ExternalInput"
    )
    d2m_d = nc.dram_tensor("d2m", [NTILES, 128, TR * F], f32, kind="ExternalOutput")
    sidx_d = nc.dram_tensor("sidx", [SCALLS, 128, L // 16], i16, kind="ExternalInput")
    spexp_d = nc.dram_tensor(
        "spexp", [SCALLS, 128, (L // 128) * 128], bf16, kind="ExternalInput"
    )
    d2s_d = nc.dram_tensor(
        "d2s", [SCALLS, 128, (L // 128) * 2], f32, kind="ExternalOutput"
    )

    # spill gather windows: idx*256B = row 2*idx, elem = 2 rows (256B)
    spill_aps = []
    for base_row, wrows in ((0, 65536 + 2), (65536, ROWS_PAD - 65536)):
        sl = tab_d[base_row : base_row + wrows, :]
        ia = sl.copy()
        ia.ap = type(ia.ap)([[128, (wrows - 2) // 2 + 1], [1, 128]])
        spill_aps.append(ia)
    scall_chunk = [0] * SPC[0] + [1] * SPC[1]

    PCs = L // 128  # 8 descs per partition per spill call

    with tile.TileContext(nc) as tc:
        with (
            tc.tile_pool(name="ix", bufs=4) as ix_pool,
            tc.tile_pool(name="sp", bufs=3) as sp_pool,
            tc.tile_pool(name="tab", bufs=4) as tab_pool,
            tc.tile_pool(name="pm", bufs=4) as pm_pool,
            tc.tile_pool(name="wk", bufs=3) as wk_pool,
            tc.tile_pool(name="out", bufs=3) as out_pool,
        ):
            # ---- spill path first (keeps Pool/queues busy early) ----
            for s in range(SCALLS):
                ch = scall_chunk[s]
                it = ix_pool.tile([128, L // 16], i16)
                nc.sync.dma_start(out=it[:], in_=sidx_d[s, :, :])
                spt = sp_pool.tile([128, PCs * 128], bf16, tag="sg")
                nc.gpsimd.dma_gather(
                    out_ap=spt[:].rearrange("p (a b) -> p a b", b=128),
                    in_ap=spill_aps[ch],
                    idxs_ap=it[:],
                    num_idxs=L,
                    num_idxs_reg=L,
                    elem_size=128,
                    elem_step=128,
                    queue_num=s % 4,
                )
                spp = sp_pool.tile([128, PCs * 128], bf16, tag="sp")
                nc.sync.dma_start(out=spp[:], in_=spexp_d[s, :, :])
                sx = sp_pool.tile([128, PCs * 128], bf16, tag="sx")
                nc.vector.tensor_tensor(
                    out=sx[:], in0=spt[:], in1=spp[:], op=mybir.AluOpType.mult
                )
                v0 = sx[:].rearrange("p (a b) -> p a b", b=64)     # [128,16,64]
                st1 = sp_pool.tile([128, PCs * 64], bf16, tag="st1")
                nc.vector.tensor_tensor(
                    out=st1[:].rearrange("p (a b) -> p a b", b=32),
                    in0=v0[:, :, 0:32],
                    in1=v0[:, :, 32:64],
                    op=mybir.AluOpType.add,
                )
                v1 = st1[:].rearrange("p (a b) -> p a b", b=32)
                st2 = sp_pool.tile([128, PCs * 32], bf16, tag="st2")
                nc.vector.tensor_tensor(
                    out=st2[:].rearrange("p (a b) -> p a b", b=16),
                    in0=v1[:, :, 0:16],
                    in1=v1[:, :, 16:32],
                    op=mybir.AluOpType.add,
                )
                v2 = st2[:].rearrange("p (a b) -> p a b", b=16)
                st3 = sp_pool.tile([128, PCs * 16], bf16, tag="st3")
                nc.vector.tensor_tensor(
                    out=st3[:].rearrange("p (a b) -> p a b", b=8),
                    in0=v2[:, :, 0:8],
                    in1=v2[:, :, 8:16],
                    op=mybir.AluOpType.add,
                )
                so = out_pool.tile([128, PCs * 2], f32, tag="so")
                nc.vector.tensor_reduce(
                    out=so[:],
                    in_=st3[:].rearrange("p (a b) -> p a b", b=8),
                    axis=mybir.AxisListType.X,
                    op=mybir.AluOpType.add,
                )
                nc.sync.dma_start(out=d2s_d[s, :, :], in_=so[:])

            # ---- main stream-expand path ----
            for T in range(NTILES):
                tt = tab_pool.tile([128, TR * C], bf16)
                nc.sync.dma_start(
                    out=tt[:],
                    in_=tab_d[T * TILE_ROWS : (T + 1) * TILE_ROWS, :].rearrange(
                        "(p l) c -> p (l c)", p=128
                    ),
                )
                pm = pm_pool.tile([128, TR * F * C], bf16)
                nc.sync.dma_start(out=pm[:], in_=pmain_d[T, :, :])
                # broadcast AP: [part, TR, F(stride 0), C]
                t3 = tt[:].rearrange("p (l c) -> p l c", c=C)
                bap = t3.copy()
                pst = t3.ap.to_list()[0]
                bap.ap = type(bap.ap)([list(pst), [C, TR], [0, F], [1, C]])
                x = wk_pool.tile([128, TR * F * C], bf16, tag="x")
                nc.vector.tensor_tensor(
                    out=x[:].rearrange("p (l f c) -> p l f c", f=F, c=C),
                    in0=bap,
                    in1=pm[:].rearrange("p (l f c) -> p l f c", f=F, c=C),
                    op=mybir.AluOpType.mult,
                )
                w0 = x[:].rearrange("p (a b) -> p a b", b=64)      # [128,96,64]
                t1 = wk_pool.tile([128, TR * F * 32], bf16, tag="t1")
                nc.vector.tensor_tensor(
                    out=t1[:].rearrange("p (a b) -> p a b", b=32),
                    in0=w0[:, :, 0:32],
                    in1=w0[:, :, 32:64],
                    op=mybir.AluOpType.add,
                )
                w1 = t1[:].rearrange("p (a b) -> p a b", b=32)
                t2 = wk_pool.tile([128, TR * F * 16], bf16, tag="t2")
                nc.vector.tensor_tensor(
                    out=t2[:].rearrange("p (a b) -> p a b", b=16),
                    in0=w1[:, :, 0:16],
                    in1=w1[:, :, 16:32],
                    op=mybir.AluOpType.add,
                )
                w2 = t2[:].rearrange("p (a b) -> p a b", b=16)
                t3s = wk_pool.tile([128, TR * F * 8], bf16, tag="t3")
                nc.vector.tensor_tensor(
                    out=t3s[:].rearrange("p (a b) -> p a b", b=8),
                    in0=w2[:, :, 0:8],
                    in1=w2[:, :, 8:16],
                    op=mybir.AluOpType.add,
                )
                ot = out_pool.tile([128, TR * F], f32, tag="mo")
                nc.vector.tensor_reduce(
                    out=ot[:],
                    in_=t3s[:].rearrange("p (a b) -> p a b", b=8),
                    axis=mybir.AxisListType.X,
                    op=mybir.AluOpType.add,
                )
                nc.sync.dma_start(out=d2m_d[T, :, :], in_=ot[:])

    nc.compile()
    _split_multi_waits(nc)
    _cached["nc"] = nc
    return nc


def _wrap16(arr):
    G, N = arr.shape
    w = arr.reshape(G, N // 16, 16).transpose(0, 2, 1)
    return np.ascontiguousarray(np.tile(w, (1, 8, 1)))


def _prep_core(flat, feats_bf, m0):
    """flat: [437500] neighbor rows in slot order. Returns device inputs +
    maps. Main: rows x F grid. Spill: rank>=F slots via 256B desc gather."""
    bf = ml_dtypes.bfloat16
    N = flat.size
    order = np.argsort(flat, kind="stable").astype(np.int64)
    srt = flat[order]
    cnt = np.bincount(flat, minlength=M_TOTAL).astype(np.int64)
    row_off = np.concatenate([[0], np.cumsum(cnt)])
    rank = np.arange(N) - row_off[srt]

    main = rank < F
    mm = np.full((ROWS_PAD, F), -1, np.int64)
    mm[srt[main], rank[main]] = order[main]

    pmain = np.zeros((ROWS_PAD * F, C), bf)
    mv = mm.ravel()
    val = mv >= 0
    pmain[val] = feats_bf[m0 + (mv[val] // K)]
    pmain = pmain.reshape(NTILES, 128, TR * F * C)

    # ---- spill ----
    sp_slots = order[~main]
    sp_rows = srt[~main]                       # sorted ascending
    sidx_all = (sp_rows // 2).astype(np.int64)
    half = (sp_rows % 2).astype(np.int64)
    chunk = (sp_rows >= 65536).astype(np.int64)
    idx_grid = np.zeros(SCALLS * L, np.int16)
    sgrid = np.full((SCALLS * L, 2), -1, np.int64)
    bounds = np.searchsorted(chunk, [0, 1, 2])
    pos = 0
    for ch in range(2):
        lo, hi = bounds[ch], bounds[ch + 1]
        n = hi - lo
        cap = SPC[ch] * L
        assert n <= cap, f"spill chunk {ch} overflow: {n} > {cap}"
        idx_grid[pos : pos + n] = (
            sidx_all[lo:hi] - 32768 * ch
        ).astype(np.int16)
        sgrid[pos + np.arange(n), half[lo:hi]] = sp_slots[lo:hi]
        pos += cap
    assert pos == SCALLS * L

    spexp = np.zeros((SCALLS * L * 2, C), bf)
    sv = sgrid.ravel()
    vals = sv >= 0
    spexp[vals] = feats_bf[m0 + (sv[vals] // K)]
    spexp = (
        spexp.reshape(SCALLS, L // 128, 128, 2 * C)
        .transpose(0, 2, 1, 3)
        .reshape(SCALLS, 128, (L // 128) * 128)
    )
    sidx = _wrap16(idx_grid.reshape(SCALLS, L))
    return sidx, spexp, pmain, mm, sgrid


def kernel(features, labels, neighbor_idx):
    from concourse.bass_utils import run_bass_kernel_spmd

    bf = ml_dtypes.bfloat16
    features = np.ascontiguousarray(np.asarray(features), dtype=np.float32)
    labels = np.asarray(labels).astype(np.int64)
    neighbor_idx = np.asarray(neighbor_idx).astype(np.int64)

    nc = _get_nc()

    feats_bf = features.astype(bf)
    tab = np.zeros((ROWS_PAD, C), bf)
    tab[:M_TOTAL] = feats_bf
    norms = (feats_bf.astype(np.float32) ** 2).sum(1)     # [100000]

    in_maps = []
    maps = []
    for c in range(N_CORES):
        m0 = c * M_CORE
        flat = neighbor_idx[m0 : m0 + M_CORE].ravel()
        sidx, spexp, pmain, mm, sgrid = _prep_core(flat, feats_bf, m0)
        maps.append((mm, sgrid, flat))
        in_maps.append(
            {"table": tab, "pmain": pmain, "sidx": sidx, "spexp": spexp}
        )
    _cached["in_maps"] = in_maps

    res = run_bass_kernel_spmd(nc, in_maps, list(range(N_CORES))).results

    posmask = (labels[:, None] == labels[neighbor_idx]).astype(np.float32)
    cnt = posmask.sum(-1)
    pm = ((cnt > 0) & (cnt < K)).astype(np.float32)

    loss_num = 0.0
    for c in range(N_CORES):
        mm, sgrid, flat = maps[c]
        m0 = c * M_CORE
        cross = np.empty(M_CORE * K, np.float32)
        d2m = res[c]["d2m"].reshape(ROWS_PAD, F)
        mv = mm.ravel()
        val = mv >= 0
        cross[mv[val]] = d2m.ravel()[val]
        d2s = (
            res[c]["d2s"]
            .reshape(SCALLS, 128, L // 128, 2)
            .transpose(0, 2, 1, 3)
            .reshape(SCALLS * L, 2)
        )
        sv = sgrid.ravel()
        vals = sv >= 0
        cross[sv[vals]] = d2s.ravel()[vals]

        slots_m = np.repeat(np.arange(M_CORE), K)
        d2_grid = (
            norms[flat] + norms[m0 + slots_m] - 2.0 * cross
        ).reshape(M_CORE, K)
        d2_grid = np.maximum(d2_grid, 0.0)

        dist = np.sqrt(d2_grid + _EPS)
        d = -dist
        d = d - d.max(axis=-1, keepdims=True)
        d = d / TEMPERATURE
        ex = np.exp(d)
        pos = (ex * posmask[m0 : m0 + M_CORE]).sum(-1)
        neg = ex.sum(-1)
        loss = -np.log(pos / neg + _EPS)
        loss_num += float((loss * pm[m0 : m0 + M_CORE]).sum())

    denom = max(float(pm.sum()), 1.0)
    return np.float32(loss_num / denom * WEIGHT)


# revision 5
# speedup vs baseline: 1.1991x; 1.1973x over previous
"""ContrastHead KNN loss on 8 TRN2 cores — v3 "stream-expand".

v2's bottleneck was GpSimd descriptor generation (~5.5 ns/desc x 134k
descs = 764 us).  v3 removes descriptors from the main path entirely:

  - Slots are sorted by neighbor row.  Each row's first F=6 slots form the
    "main" grid [100352 rows x 6]; the device STREAMS the bf16 table
    sequentially (12.8 MB, large descriptors, no SWDGE) and expands each
    row to its 6 slots with a stride-0 broadcast AP on the DVE.
  - d2 = |g|^2 + |p|^2 - 2 g.p: the device computes only the cross term
    (mult + bf16 half-tree + f32 reduce); row norms are added on the host.
  - Per-slot point rows are staged by the host as a sequential bf16 stream.
  - Overflow slots (rank >= 6, ~29k/core = 6.6%) go through a small
    dma_gather spill path (256B descs on the plain bf16 table, 2 chunks).
  - Host: un-permute, add norms, sqrt/softmax/mask (O(M*K) numpy).
"""
import numpy as np
import ml_dtypes

M_TOTAL = 100000
C = 64
K = 35
N_CORES = 8
M_CORE = M_TOTAL // N_CORES          # 12500

F = 6                                # padded slots per row (main path)
TR = 16                              # rows per partition per tile
TILE_ROWS = 128 * TR                 # 2048
NTILES = 49
ROWS_PAD = NTILES * TILE_ROWS        # 100352

L = 1024                             # idx per spill gather call
SPC = [21, 12]                       # spill calls per chunk (rows <65536, >=)
SCALLS = sum(SPC)                    # 33

_EPS = 1e-7
TEMPERATURE = 0.1
WEIGHT = 1.0

_cached = {}


def _get_nc():
    if "nc" in _cached:
        return _cached["nc"]
    import concourse.bacc as bacc
    import concourse.mybir as mybir
    import concourse.tile as tile
    import bass_rust
    from concourse.vector_clock import ScopedClock

    def _patched_drain_and_barrier(self, tick_clock, wait_clock):
        holder = self.nc.sync.nop(nofuse=True, hint="tile_exit_waits")
        wait_clock.add_sem_waits(
            holder.ins, ScopedClock({None: tick_clock.global_clock})
        )
        si = holder.ins.sync_info
        waits = list(si.on_wait) if si is not None else []
        if len(waits) > 1:
            si.on_wait[:] = waits[:1]
            for w in waits[1:]:
                nop = self.nc.sync.nop(nofuse=True, hint="tile_exit_waits")
                nop.ins.sync_info = mybir.SyncInfo(on_wait=[w], on_update=[])
        self.nc.sync.drain()
        self.nc.all_engine_barrier()
        assert self.sems is not None
        popped = self.nc._tile_sem_poison_stack.pop()
        assert popped is self._sem_poison
        self.nc.clear_and_free_semaphores(list(self.sems.allocated().values()))
        self.nc.all_engine_barrier()

    tile.TileContext._drain_and_barrier = _patched_drain_and_barrier

    def _split_multi_waits(nc, limit=1):
        counter = [0]
        for func in nc.m.functions:
            for bb in func.blocks:
                out = []
                changed = False
                for inst in bb.instructions:
                    si = inst.sync_info
                    waits = list(si.on_wait) if si is not None else []
                    if len(waits) > limit:
                        for w in waits[:-limit]:
                            nop = bass_rust.InstNoOp(
                                name=f"waitsplit-nop-{counter[0]}", ins=[], outs=[]
                            )
                            counter[0] += 1
                            nop.engine = inst.engine
                            nop.sync_info = mybir.SyncInfo(on_wait=[w], on_update=[])
                            nop.bass_nofuse = True
                            out.append(nop)
                        inst.sync_info = mybir.SyncInfo(
                            on_wait=waits[-limit:], on_update=list(si.on_update)
                        )
                        changed = True
                    out.append(inst)
                if changed:
                    bb.instructions = out

    nc = bacc.Bacc(
        "TRN2", target_bir_lowering=False, debug=False, num_swdge_queues=4
    )
    f32 = mybir.dt.float32
    bf16 = mybir.dt.bfloat16
    i16 = mybir.dt.int16

    tab_d = nc.dram_tensor("table", [ROWS_PAD, C], bf16, kind="ExternalInput")
    pmain_d = nc.dram_tensor(
        "pmain", [NTILES, 128, TR * F * C], bf16, kind="ExternalInput"
    )
    d2m_d = nc.dram_tensor("d2m", [NTILES, 128, TR * F], f32, kind="ExternalOutput")
    sidx_d = nc.dram_tensor("sidx", [SCALLS, 128, L // 16], i16, kind="ExternalInput")
    spexp_d = nc.dram_tensor(
        "spexp", [SCALLS, 128, (L // 128) * 128], bf16, kind="ExternalInput"
    )
    d2s_d = nc.dram_tensor(
        "d2s", [SCALLS, 128, (L // 128) * 2], f32, kind="ExternalOutput"
    )

    # spill gather windows: idx*256B = row 2*idx, elem = 2 rows (256B)
    spill_aps = []
    for base_row, wrows in ((0, 65536 + 2), (65536, ROWS_PAD - 65536)):
        sl = tab_d[base_row : base_row + wrows, :]
        ia = sl.copy()
        ia.ap = type(ia.ap)([[128, (wrows - 2) // 2 + 1], [1, 128]])
        spill_aps.append(ia)
    scall_chunk = [0] * SPC[0] + [1] * SPC[1]

    PCs = L // 128  # 8 descs per partition per spill call

    with tile.TileContext(nc) as tc:
        with (
            tc.tile_pool(name="ix", bufs=4) as ix_pool,
            tc.tile_pool(name="sp", bufs=3) as sp_pool,
            tc.tile_pool(name="tab", bufs=4) as tab_pool,
            tc.tile_pool(name="pm", bufs=4) as pm_pool,
            tc.tile_pool(name="wk", bufs=3) as wk_pool,
            tc.tile_pool(name="out", bufs=3) as out_pool,
        ):
            def emit_spill(s):
                ch = scall_chunk[s]
                it = ix_pool.tile([128, L // 16], i16)
                nc.sync.dma_start(out=it[:], in_=sidx_d[s, :, :])
                spt = sp_pool.tile([128, PCs * 128], bf16, tag="sg")
                nc.gpsimd.dma_gather(
                    out_ap=spt[:].rearrange("p (a b) -> p a b", b=128),
                    in_ap=spill_aps[ch],
                    idxs_ap=it[:],
                    num_idxs=L,
                    num_idxs_reg=L,
                    elem_size=128,
                    elem_step=128,
                    queue_num=s % 4,
                )
                spp = sp_pool.tile([128, PCs * 128], bf16, tag="sp")
                nc.sync.dma_start(out=spp[:], in_=spexp_d[s, :, :])
                sx = sp_pool.tile([128, PCs * 128], bf16, tag="sx")
                nc.vector.tensor_tensor(
                    out=sx[:], in0=spt[:], in1=spp[:], op=mybir.AluOpType.mult
                )
                v0 = sx[:].rearrange("p (a b) -> p a b", b=64)     # [128,16,64]
                st1 = sp_pool.tile([128, PCs * 64], bf16, tag="st1")
                nc.vector.tensor_tensor(
                    out=st1[:].rearrange("p (a b) -> p a b", b=32),
                    in0=v0[:, :, 0:32],
                    in1=v0[:, :, 32:64],
                    op=mybir.AluOpType.add,
                )
                v1 = st1[:].rearrange("p (a b) -> p a b", b=32)
                st2 = sp_pool.tile([128, PCs * 32], bf16, tag="st2")
                nc.vector.tensor_tensor(
                    out=st2[:].rearrange("p (a b) -> p a b", b=16),
                    in0=v1[:, :, 0:16],
                    in1=v1[:, :, 16:32],
                    op=mybir.AluOpType.add,
                )
                v2 = st2[:].rearrange("p (a b) -> p a b", b=16)
                st3 = sp_pool.tile([128, PCs * 16], bf16, tag="st3")
                nc.vector.tensor_tensor(
                    out=st3[:].rearrange("p (a b) -> p a b", b=8),
                    in0=v2[:, :, 0:8],
                    in1=v2[:, :, 8:16],
                    op=mybir.AluOpType.add,
                )
                so = out_pool.tile([128, PCs * 2], f32, tag="so")
                nc.vector.tensor_reduce(
                    out=so[:],
                    in_=st3[:].rearrange("p (a b) -> p a b", b=8),
                    axis=mybir.AxisListType.X,
                    op=mybir.AluOpType.add,
                )
                nc.sync.dma_start(out=d2s_d[s, :, :], in_=so[:])

            def emit_main(T):
                tt = tab_pool.tile([128, TR * C], bf16)
                nc.sync.dma_start(
                    out=tt[:],
                    in_=tab_d[T * TILE_ROWS : (T + 1) * TILE_ROWS, :].rearrange(
                        "(p l) c -> p (l c)", p=128
                    ),
                )
                pm = pm_pool.tile([128, TR * F * C], bf16)
                nc.sync.dma_start(out=pm[:], in_=pmain_d[T, :, :])
                # broadcast AP: [part, TR, F(stride 0), C]
                t3 = tt[:].rearrange("p (l c) -> p l c", c=C)
                bap = t3.copy()
                pst = t3.ap.to_list()[0]
                bap.ap = type(bap.ap)([list(pst), [C, TR], [0, F], [1, C]])
                x = wk_pool.tile([128, TR * F * C], bf16, tag="x")
                nc.vector.tensor_tensor(
                    out=x[:].rearrange("p (l f c) -> p l f c", f=F, c=C),
                    in0=bap,
                    in1=pm[:].rearrange("p (l f c) -> p l f c", f=F, c=C),
                    op=mybir.AluOpType.mult,
                )
                w0 = x[:].rearrange("p (a b) -> p a b", b=64)      # [128,96,64]
                t1 = wk_pool.tile([128, TR * F * 32], bf16, tag="t1")
                nc.vector.tensor_tensor(
                    out=t1[:].rearrange("p (a b) -> p a b", b=32),
                    in0=w0[:, :, 0:32],
                    in1=w0[:, :, 32:64],
                    op=mybir.AluOpType.add,
                )
                w1 = t1[:].rearrange("p (a b) -> p a b", b=32)
                t2 = wk_pool.tile([128, TR * F * 16], bf16, tag="t2")
                nc.vector.tensor_tensor(
                    out=t2[:].rearrange("p (a b) -> p a b", b=16),
                    in0=w1[:, :, 0:16],
                    in1=w1[:, :, 16:32],
                    op=mybir.AluOpType.add,
                )
                w2 = t2[:].rearrange("p (a b) -> p a b", b=16)
                t3s = wk_pool.tile([128, TR * F * 8], bf16, tag="t3")
                nc.vector.tensor_tensor(
                    out=t3s[:].rearrange("p (a b) -> p a b", b=8),
                    in0=w2[:, :, 0:8],
                    in1=w2[:, :, 8:16],
                    op=mybir.AluOpType.add,
                )
                ot = out_pool.tile([128, TR * F], f32, tag="mo")
                nc.vector.tensor_reduce(
                    out=ot[:],
                    in_=t3s[:].rearrange("p (a b) -> p a b", b=8),
                    axis=mybir.AxisListType.X,
                    op=mybir.AluOpType.add,
                )
                nc.sync.dma_start(out=d2m_d[T, :, :], in_=ot[:])

            si = 0
            for T in range(NTILES):
                if si < SCALLS and T * SCALLS >= si * NTILES:
                    emit_spill(si)
                    si += 1
                emit_main(T)
            while si < SCALLS:
                emit_spill(si)
                si += 1

    nc.compile()
    _split_multi_waits(nc)
    _cached["nc"] = nc
    return nc


def _wrap16(arr):
    G, N = arr.shape
    w = arr.reshape(G, N // 16, 16).transpose(0, 2, 1)
    return np.ascontiguousarray(np.tile(w, (1, 8, 1)))


def _prep_core(flat, feats_bf, m0):
    """flat: [437500] neighbor rows in slot order. Returns device inputs +
    maps. Main: rows x F grid. Spill: rank>=F slots via 256B desc gather."""
    bf = ml_dtypes.bfloat16
    N = flat.size
    order = np.argsort(flat, kind="stable").astype(np.int64)
    srt = flat[order]
    cnt = np.bincount(flat, minlength=M_TOTAL).astype(np.int64)
    row_off = np.concatenate([[0], np.cumsum(cnt)])
    rank = np.arange(N) - row_off[srt]

    main = rank < F
    mm = np.full((ROWS_PAD, F), -1, np.int64)
    mm[srt[main], rank[main]] = order[main]

    pmain = np.zeros((ROWS_PAD * F, C), bf)
    mv = mm.ravel()
    val = mv >= 0
    pmain[val] = feats_bf[m0 + (mv[val] // K)]
    pmain = pmain.reshape(NTILES, 128, TR * F * C)

    # ---- spill ----
    sp_slots = order[~main]
    sp_rows = srt[~main]                       # sorted ascending
    sidx_all = (sp_rows // 2).astype(np.int64)
    half = (sp_rows % 2).astype(np.int64)
    chunk = (sp_rows >= 65536).astype(np.int64)
    idx_grid = np.zeros(SCALLS * L, np.int16)
    sgrid = np.full((SCALLS * L, 2), -1, np.int64)
    bounds = np.searchsorted(chunk, [0, 1, 2])
    pos = 0
    for ch in range(2):
        lo, hi = bounds[ch], bounds[ch + 1]
        n = hi - lo
        cap = SPC[ch] * L
        assert n <= cap, f"spill chunk {ch} overflow: {n} > {cap}"
        idx_grid[pos : pos + n] = (
            sidx_all[lo:hi] - 32768 * ch
        ).astype(np.int16)
        sgrid[pos + np.arange(n), half[lo:hi]] = sp_slots[lo:hi]
        pos += cap
    assert pos == SCALLS * L

    spexp = np.zeros((SCALLS * L * 2, C), bf)
    sv = sgrid.ravel()
    vals = sv >= 0
    spexp[vals] = feats_bf[m0 + (sv[vals] // K)]
    spexp = (
        spexp.reshape(SCALLS, L // 128, 128, 2 * C)
        .transpose(0, 2, 1, 3)
        .reshape(SCALLS, 128, (L // 128) * 128)
    )
    sidx = _wrap16(idx_grid.reshape(SCALLS, L))
    return sidx, spexp, pmain, mm, sgrid


def kernel(features, labels, neighbor_idx):
    from concourse.bass_utils import run_bass_kernel_spmd

    bf = ml_dtypes.bfloat16
    features = np.ascontiguousarray(np.asarray(features), dtype=np.float32)
    labels = np.asarray(labels).astype(np.int64)
    neighbor_idx = np.asarray(neighbor_idx).astype(np.int64)

    nc = _get_nc()

    feats_bf = features.astype(bf)
    tab = np.zeros((ROWS_PAD, C), bf)
    tab[:M_TOTAL] = feats_bf
    norms = (feats_bf.astype(np.float32) ** 2).sum(1)     # [100000]

    in_maps = []
    maps = []
    for c in range(N_CORES):
        m0 = c * M_CORE
        flat = neighbor_idx[m0 : m0 + M_CORE].ravel()
        sidx, spexp, pmain, mm, sgrid = _prep_core(flat, feats_bf, m0)
        maps.append((mm, sgrid, flat))
        in_maps.append(
            {"table": tab, "pmain": pmain, "sidx": sidx, "spexp": spexp}
        )
    _cached["in_maps"] = in_maps

    res = run_bass_kernel_spmd(nc, in_maps, list(range(N_CORES))).results

    posmask = (labels[:, None] == labels[neighbor_idx]).astype(np.float32)
    cnt = posmask.sum(-1)
    pm = ((cnt > 0) & (cnt < K)).astype(np.float32)

    loss_num = 0.0
    for c in range(N_CORES):
        mm, sgrid, flat = maps[c]
        m0 = c * M_CORE
        cross = np.empty(M_CORE * K, np.float32)
        d2m = res[c]["d2m"].reshape(ROWS_PAD, F)
        mv = mm.ravel()
        val = mv >= 0
        cross[mv[val]] = d2m.ravel()[val]
        d2s = (
            res[c]["d2s"]
            .reshape(SCALLS, 128, L // 128, 2)
            .transpose(0, 2, 1, 3)
            .reshape(SCALLS * L, 2)
        )
        sv = sgrid.ravel()
        vals = sv >= 0
        cross[sv[vals]] = d2s.ravel()[vals]

        slots_m = np.repeat(np.arange(M_CORE), K)
        d2_grid = (
            norms[flat] + norms[m0 + slots_m] - 2.0 * cross
        ).reshape(M_CORE, K)
        d2_grid = np.maximum(d2_grid, 0.0)

        dist = np.sqrt(d2_grid + _EPS)
        d = -dist
        d = d - d.max(axis=-1, keepdims=True)
        d = d / TEMPERATURE
        ex = np.exp(d)
        pos = (ex * posmask[m0 : m0 + M_CORE]).sum(-1)
        neg = ex.sum(-1)
        loss = -np.log(pos / neg + _EPS)
        loss_num += float((loss * pm[m0 : m0 + M_CORE]).sum())

    denom = max(float(pm.sum()), 1.0)
    return np.float32(loss_num / denom * WEIGHT)
